# revision 1
# baseline (speedup 1.0000x reference)
"""AutoCorrelation attention kernel for 8 Trainium2 NeuronCores.

Strategy (per sharding hint): pure data-parallel over batch B=8 -> one batch
element per NeuronCore; zero inter-core communication.

All device math is expressed with ops that lower robustly on the Neuron
backend (matmul / elementwise / reductions / select):
  - rfft/irfft are dense DFT matmuls with host-precomputed cos/sin matrices
    (L=2048, 1025 rfft bins) on the TensorEngine.
  - top-k (k=15) per channel is a 15-round iterative max+mask to find the
    15th-largest threshold, then a dense softmax over the thresholded set
    (order-invariant, identical to softmax over top_k values).
  - the roll-gather aggregation sum_j w_j * roll(v, -I_j) is a circular
    cross-correlation with the sparse weight signal s (s[I_j]=w_j), computed
    in the frequency domain: A = irfft(rfft(v) * conj(rfft(s))).
"""

import functools
import math

import numpy as np
import jax
import jax.numpy as jnp

B, L, DM, D = 8, 2048, 512, 512
K_TOP = int(math.floor(2 * math.log(L)))  # 15
NF = L // 2 + 1  # 1025 rfft bins


def _dft_mats():
    t = np.arange(L, dtype=np.float64)
    f = np.arange(NF, dtype=np.float64)
    ang = 2.0 * np.pi * np.outer(f, t) / L  # [NF, L]
    cre = np.cos(ang).astype(np.float32)          # Xre = CRE @ x
    cim = (-np.sin(ang)).astype(np.float32)       # Xim = CIM @ x
    w = np.full(NF, 2.0, dtype=np.float64)
    w[0] = 1.0
    w[-1] = 1.0
    angi = 2.0 * np.pi * np.outer(t, f) / L  # [L, NF]
    gre = (np.cos(angi) * w / L).astype(np.float32)     # x = GRE@Pre + GIM@Pim
    gim = (-np.sin(angi) * w / L).astype(np.float32)
    return cre, cim, gre, gim


_CRE, _CIM, _GRE, _GIM = _dft_mats()


def _one_batch(Qb, Kb, Vb, Wq, bq, Wk, bk, Wv, bv, cre, cim, gre, gim):
    # Dense projections [L, D]
    q = Qb @ Wq + bq
    k = Kb @ Wk + bk
    v = Vb @ Wv + bv
    # rfft via DFT matmuls
    qfr = cre @ q
    qfi = cim @ q
    kfr = cre @ k
    kfi = cim @ k
    # P = Qf * conj(Kf)
    pr = qfr * kfr + qfi * kfi
    pi = qfi * kfr - qfr * kfi
    # Rxx = irfft(P)  [L, D] (lag, channel)
    r = gre @ pr + gim @ pi
    # threshold = 15th largest per channel (iterative max + mask-out)
    rw = r
    m0 = None
    mj = None
    for j in range(K_TOP):
        mj = jnp.max(rw, axis=0)  # [D]
        if j == 0:
            m0 = mj
        rw = jnp.where(rw >= mj[None, :], -jnp.inf, rw)
    mask = r >= mj[None, :]
    e = jnp.where(mask, jnp.exp(r - m0[None, :]), 0.0)
    s = e / jnp.sum(e, axis=0, keepdims=True)  # sparse softmax signal [L, D]
    # A = irfft(rfft(v) * conj(rfft(s)))  -> sum_j w_j v[(t+I_j) % L]
    vfr = cre @ v
    vfi = cim @ v
    sfr = cre @ s
    sfi = cim @ s
    ar = vfr * sfr + vfi * sfi
    ai = vfi * sfr - vfr * sfi
    return gre @ ar + gim @ ai  # [L, D]


def kernel(Q, K, V, WQ_w, WQ_b, WK_w, WK_b, WV_w, WV_b):
    devs = jax.devices()[:B]
    fn = jax.jit(_one_batch)
    shared_host = (WQ_w, WQ_b, WK_w, WK_b, WV_w, WV_b,
                   _CRE, _CIM, _GRE, _GIM)
    outs = []
    for b, dev in enumerate(devs):
        args = [jax.device_put(np.asarray(Q[b]), dev),
                jax.device_put(np.asarray(K[b]), dev),
                jax.device_put(np.asarray(V[b]), dev)]
        args += [jax.device_put(np.asarray(a), dev) for a in shared_host]
        outs.append(fn(*args))  # async dispatch; one core per batch element
    return np.stack([np.asarray(o) for o in outs]).astype(np.float32)
